# revision 4
# baseline (speedup 1.0000x reference)
"""Trainium2 Bass kernel for Llama-style GQA attention (nn_LlamaAttention) — v2.

Shapes (hardcoded): hidden_states [2, 2048, 4096] f32,
attention_mask [2, 1, 2048, 2048] f32, position_ids [2, 2048] i64,
Wq [4096, 4096], Wk/Wv [4096, 1024], Wo [4096, 4096].

Sharding: tensor-parallel over heads across 8 NeuronCores. Core c owns
Q heads 4c..4c+3 and KV head c; host sums the 8 full-shape partials.

v2 changes vs v1 (driven by timeline-sim evidence: DMA-bound phase B,
exp-stalled phase C):
  - Weights SBUF-resident, loaded via a handful of large batched DMAs
    (rearrange APs); xt streamed in quarter-chunk 1MB DMAs. ~70 DMAs/rep
    instead of ~1300 (HWDGE fixed cost 625ns/DMA dominated the old plan).
  - bf16 trig + bf16 PSUM-evict staging; RoPE in 4 DVE ops (3 tile-volumes)
    using a host-precomputed sign-swapped sin layout.
  - Causal diagonal blocks run at reduced width (cols >= 128j only) and the
    triangle mask is ADDED in PSUM via an identity-matmul with a shared
    128x128 additive tile; general masks use the same additive path.
  - Phase C software-pipelined with lookahead 2 (3 score PSUM banks) so the
    ScalarE exp latency hides under the next blocks' score matmuls.
  - Phase D PSUM evictions staged into a [128, 4096] tile, one 2MB DMA per
    token tile; evictions alternate ScalarE/DVE.
"""
import sys
sys.path.insert(0, "/opt/trn_rl_repo")
import numpy as np

import concourse.bass as bass
import concourse.bacc as bacc
import concourse.mybir as mybir
import concourse.tile as tile
import ml_dtypes

F32 = mybir.dt.float32
BF16 = mybir.dt.bfloat16
AF = mybir.ActivationFunctionType
ALU = mybir.AluOpType

H = 4096
NH = 32
NKV = 8
D = 128
B = 2
S = 2048
T = B * S
NC = 8
HQ = NH // NC          # 4 q heads per core
QCOLS = HQ * D         # 512
ROPE_BASE = 10000.0
NQC = S // 512         # 4 q-chunks of 512 per batch
NKT = S // 128         # 16 k-tiles of 128 per batch
NCH = T // 512         # 8 projection chunks
MASK_PRELOAD_MAX = 24
NEG = -100.0           # additive mask value for "fully masked"

# block kinds in plan
FREE, DIAG, GEN = 0, 1, 2


def _build_program(plan, n_uniq, nreps, trig_len=S, phases=(1, 1, 1)):
    """plan[b][qc] = tuple of (kt, kind, arg); arg = col offset for DIAG,
    mask index for GEN, 0 for FREE."""
    nc = bacc.Bacc(None, target_bir_lowering=False)

    xt_d = nc.dram_tensor("xt", [H, T], BF16, kind="ExternalInput")
    wq_d = nc.dram_tensor("wq", [H, QCOLS], BF16, kind="ExternalInput")
    wkv_d = nc.dram_tensor("wkv", [H, 2 * D], BF16, kind="ExternalInput")
    wo_d = nc.dram_tensor("wo", [QCOLS, H], BF16, kind="ExternalInput")
    cosq_d = nc.dram_tensor("cosq", [D, trig_len], BF16, kind="ExternalInput")
    ssq_d = nc.dram_tensor("ssq", [D, trig_len], BF16, kind="ExternalInput")
    cosk_d = nc.dram_tensor("cosk", [D, trig_len], BF16, kind="ExternalInput")
    ssk_d = nc.dram_tensor("ssk", [D, trig_len], BF16, kind="ExternalInput")
    nmask = max(n_uniq, 1)
    masks_d = nc.dram_tensor("masks", [nmask, 128, 512], BF16, kind="ExternalInput")
    trimask_d = nc.dram_tensor("trimask", [128, 128], BF16, kind="ExternalInput")
    onescol_d = nc.dram_tensor("onescol", [128, 1], BF16, kind="ExternalInput")
    onesrow_d = nc.dram_tensor("onesrow", [2, 128], BF16, kind="ExternalInput")
    ident_d = nc.dram_tensor("ident", [128, 128], BF16, kind="ExternalInput")
    out_d = nc.dram_tensor("out", [T, H], BF16, kind="ExternalOutput")

    preload_masks = 0 < n_uniq <= MASK_PRELOAD_MAX

    def trig_off(chunk):
        return (chunk * 512) % trig_len

    with tile.TileContext(nc) as tc:

        def phase_b(qt, ktr, vsb, idt):
            with tc.tile_pool(name="wres", bufs=1) as wres, \
                 tc.tile_pool(name="trig", bufs=1) as trigp, \
                 tc.tile_pool(name="xtp", bufs=3) as xtp, \
                 tc.tile_pool(name="pbps", bufs=1, space="PSUM") as pbps, \
                 tc.tile_pool(name="tps", bufs=2, space="PSUM") as tps, \
                 tc.tile_pool(name="stg", bufs=2) as stg, \
                 tc.tile_pool(name="vbfp", bufs=2) as vbfp, \
                 tc.tile_pool(name="rope", bufs=2) as ropep:
                wq_r = wres.tile([128, 32, QCOLS], BF16, name="wq_r")
                wkv_r = wres.tile([128, 32, 2 * D], BF16, name="wkv_r")
                wq_src = wq_d[:, :].rearrange("(kt p) c -> p kt c", p=128)
                cq = trigp.tile([128, trig_len], BF16, name="cq")
                sq = trigp.tile([128, trig_len], BF16, name="sq")
                ck = trigp.tile([128, trig_len], BF16, name="ck")
                sk = trigp.tile([128, trig_len], BF16, name="sk")

                def load_residents_late():
                    # emitted after chunk-0 xq DMAs: the in-order DMA queue
                    # then delivers (xq0, wq g0) first so the first matmul
                    # starts ~6us in instead of ~25us.
                    for g in range(1, 4):
                        nc.sync.dma_start(wq_r[:, g * 8:(g + 1) * 8, :],
                                          wq_src[:, g * 8:(g + 1) * 8, :])
                    nc.sync.dma_start(
                        wkv_r[:, :, :],
                        wkv_d[:, :].rearrange("(kt p) c -> p kt c", p=128))
                    nc.sync.dma_start(cq[:], cosq_d[:])
                    nc.sync.dma_start(sq[:], ssq_d[:])
                    nc.sync.dma_start(ck[:], cosk_d[:])
                    nc.sync.dma_start(sk[:], ssk_d[:])

                xt_src = xt_d[:, :].rearrange("(kt p) t -> p kt t", p=128)
                pend_tp = []   # deferred V transposes: (vbf_tile, chunk)

                def emit_transposes():
                    while pend_tp:
                        vbf, n0 = pend_tp.pop(0)
                        for j in range(4):
                            ktg = 4 * (n0 % 4) + j
                            tp_t = tps.tile([128, 128], BF16, tag="tp", name="tp_t")
                            nc.tensor.transpose(
                                tp_t[:], vbf[:, j * 128:(j + 1) * 128], idt[:])
                            nc.scalar.copy(
                                vsb[n0 // 4][:, ktg * 128:(ktg + 1) * 128],
                                tp_t[:])

                for n in range(NCH):
                    tok = slice(n * 512, (n + 1) * 512)
                    nb = n // 4
                    tokl = slice((n % 4) * 512, (n % 4) * 512 + 512)
                    to = trig_off(n)
                    # quarter i covers H-row tiles 8i..8i+7 for this token chunk
                    xq = [xtp.tile([128, 8, 512], BF16, tag="xq",
                                   name=f"xq{i}") for i in range(4)]
                    if n == 0:
                        # fine-grained first transfers: k-tiles 0-1 land in
                        # ~1.6us so the first matmuls start almost immediately
                        nc.sync.dma_start(xq[0][:, 0:2, :],
                                          xt_src[:, 0:2, tok])
                        nc.sync.dma_start(wq_r[:, 0:2, :], wq_src[:, 0:2, :])
                        nc.sync.dma_start(xq[0][:, 2:8, :],
                                          xt_src[:, 2:8, tok])
                        nc.sync.dma_start(wq_r[:, 2:8, :], wq_src[:, 2:8, :])
                        for i in range(1, 4):
                            nc.sync.dma_start(xq[i][:, :, :],
                                              xt_src[:, 8 * i:8 * i + 8, tok])
                        load_residents_late()
                    else:
                        for i in range(4):
                            nc.sync.dma_start(xq[i][:, :, :],
                                              xt_src[:, 8 * i:8 * i + 8, tok])
                    ps_q = [pbps.tile([128, 512], F32, tag=f"pq{m}",
                                      name=f"psq{m}") for m in range(HQ)]
                    ps_k = pbps.tile([128, 512], F32, tag="pk", name="psk")
                    ps_v = pbps.tile([128, 512], F32, tag="pv", name="psv")
                    for k in range(32):
                        rhs = xq[k // 8][:, k % 8, :]
                        st = (k == 0)
                        sp = (k == 31)
                        for m in range(HQ):
                            nc.tensor.matmul(
                                ps_q[m][:], wq_r[:, k, m * 128:(m + 1) * 128],
                                rhs, start=st, stop=sp)
                        nc.tensor.matmul(ps_k[:], wkv_r[:, k, 0:D], rhs,
                                         start=st, stop=sp)
                        nc.tensor.matmul(ps_v[:], wkv_r[:, k, D:2 * D], rhs,
                                         start=st, stop=sp)

                    # previous chunk's V transposes ride behind this chunk's MMs
                    emit_transposes()

                    # PSUM -> bf16 staging on ScalarE
                    stq = [stg.tile([128, 512], BF16, tag=f"sq{m}",
                                    name=f"stq{m}") for m in range(HQ)]
                    stk = stg.tile([128, 512], BF16, tag="sk", name="stk")
                    vbf = vbfp.tile([128, 512], BF16, tag="vbf", name="vbf")
                    for m in range(HQ):
                        nc.scalar.copy(stq[m][:], ps_q[m][:])
                    nc.scalar.copy(stk[:], ps_k[:])
                    nc.scalar.copy(vbf[:], ps_v[:])
                    pend_tp.append((vbf, n))

                    # RoPE: out = x*cos + swap(x)*ss   (4 DVE ops)
                    # ss layout: top half = +sin, bottom half = -sin, so each
                    # half-TT reads in0/in1 at the SAME base partition (HW
                    # requires equal base partitions for two SBUF inputs);
                    # the sin halves are identical so this is exact.
                    def rope(dst, x, cos_t, ss_t, nm):
                        rot = ropep.tile([128, 512], BF16, tag="rot",
                                         name=f"rot{nm}")
                        nc.vector.tensor_tensor(
                            rot[0:64, :], x[64:128, :],
                            ss_t[64:128, to:to + 512], ALU.mult)
                        nc.vector.tensor_tensor(
                            rot[64:128, :], x[0:64, :],
                            ss_t[0:64, to:to + 512], ALU.mult)
                        t1 = ropep.tile([128, 512], BF16, tag="t1",
                                        name=f"t1{nm}")
                        nc.vector.tensor_tensor(
                            t1[:], x[:], cos_t[:, to:to + 512], ALU.mult)
                        nc.vector.tensor_tensor(dst, t1[:], rot[:], ALU.add)

                    for m in range(HQ):
                        rope(qt[m][nb][:, tokl], stq[m][:], cq, sq, f"q{m}")
                    rope(ktr[nb][:, tokl], stk[:], ck, sk, "k")

                emit_transposes()

        def phase_c(qt, ktr, vsb, onc, onr, idt, aot):
            with tc.tile_pool(name="scps", bufs=3, space="PSUM") as scps, \
                 tc.tile_pool(name="aops", bufs=2, space="PSUM") as aops, \
                 tc.tile_pool(name="dnps", bufs=2, space="PSUM") as dnps, \
                 tc.tile_pool(name="bcps", bufs=1, space="PSUM") as bcps, \
                 tc.tile_pool(name="atp", bufs=4) as atp, \
                 tc.tile_pool(name="trim", bufs=1) as trimp, \
                 tc.tile_pool(name="mskp", bufs=1 if preload_masks else 4) as mskp, \
                 tc.tile_pool(name="rcp", bufs=2) as rcp:
                tri = trimp.tile([128, 128], BF16, name="tri")
                nc.sync.dma_start(tri[:], trimask_d[:])
                if preload_masks:
                    mres = mskp.tile([128, n_uniq * 512], BF16, name="mres")
                    for u in range(n_uniq):
                        nc.sync.dma_start(mres[:, u * 512:(u + 1) * 512],
                                          masks_d[u])

                pending = []   # deferred per-(b,h,qc) normalizations

                def make_norm(ao_t, dn_t, h, bb, qs0):
                    def norm():
                        rc = rcp.tile([1, 512], F32, tag="rc", name="rc")
                        nc.vector.reciprocal(rc[:], dn_t[:])
                        rhi = rcp.tile([1, 512], BF16, tag="rhi", name="rhi")
                        nc.vector.tensor_copy(rhi[:], rc[:])
                        rlo = rcp.tile([1, 512], BF16, tag="rlo", name="rlo")
                        nc.vector.scalar_tensor_tensor(
                            rlo[:], rc[:], 1.0, rhi[:],
                            ALU.mult, ALU.subtract)
                        bc_t = bcps.tile([128, 512], F32, tag="bc",
                                         name="bc_t")
                        nc.tensor.matmul(bc_t[:], onr[0:1, :], rhi[:],
                                         start=True, stop=False)
                        nc.tensor.matmul(bc_t[:], onr[0:1, :], rlo[:],
                                         start=False, stop=True)
                        bc_sb = rcp.tile([128, 512], F32, tag="bcsb",
                                         name="bc_sb")
                        nc.vector.tensor_copy(bc_sb[:], bc_t[:])
                        nc.vector.tensor_tensor(
                            aot[h][bb][:, qs0:qs0 + 512], ao_t[:],
                            bc_sb[:], ALU.mult)
                    return norm

                def flush():
                    while pending:
                        pending.pop(0)()

                for b in range(B):
                    for h in range(HQ):
                        for qc in range(NQC):
                            qs0 = qc * 512
                            blocks = plan[b][qc]
                            if len(blocks) == 0:
                                nc.vector.memset(
                                    aot[h][b][:, qs0:qs0 + 512], 0.0)
                                continue
                            last = len(blocks) - 1
                            ao_t = aops.tile([128, 512], F32, tag="ao",
                                             name="ao_t")
                            dn_t = dnps.tile([1, 512], F32, tag="dn",
                                             name="dn_t")
                            sc_tiles = [None] * len(blocks)
                            at_tiles = [None] * len(blocks)
                            offs = [None] * len(blocks)

                            def emit_scores(i):
                                kt, kind, arg = blocks[i]
                                o = arg if kind == DIAG else 0
                                offs[i] = o
                                ks = slice(kt * 128, kt * 128 + 128)
                                sc_t = scps.tile([128, 512], F32, tag="sc",
                                                 name="sc_t")
                                sc_tiles[i] = sc_t
                                if kind == FREE:
                                    nc.tensor.matmul(
                                        sc_t[:], ktr[b][:, ks],
                                        qt[h][b][:, qs0:qs0 + 512],
                                        start=True, stop=True)
                                elif kind == DIAG:
                                    nc.tensor.matmul(
                                        sc_t[:, o:512], ktr[b][:, ks],
                                        qt[h][b][:, qs0 + o:qs0 + 512],
                                        start=True, stop=False)
                                    nc.tensor.matmul(
                                        sc_t[:, o:o + 128], idt[:], tri[:],
                                        start=False, stop=True)
                                else:  # GEN
                                    nc.tensor.matmul(
                                        sc_t[:], ktr[b][:, ks],
                                        qt[h][b][:, qs0:qs0 + 512],
                                        start=True, stop=False)
                                    if preload_masks:
                                        mt = mres[:, arg * 512:(arg + 1) * 512]
                                    else:
                                        mt_t = mskp.tile(
                                            [128, 512], BF16, tag="mst",
                                            name="mst")
                                        nc.sync.dma_start(mt_t[:], masks_d[arg])
                                        mt = mt_t[:]
                                    nc.tensor.matmul(
                                        sc_t[:], idt[:], mt,
                                        start=False, stop=True)
                                at = atp.tile([128, 512], BF16, tag="at",
                                              name="at")
                                at_tiles[i] = at
                                nc.scalar.activation(
                                    at[:, o:512], sc_t[:, o:512], AF.Exp)

                            def emit_avdn(i):
                                kt, kind, arg = blocks[i]
                                o = offs[i]
                                at = at_tiles[i]
                                nc.tensor.matmul(
                                    ao_t[:, o:512],
                                    vsb[b][:, kt * 128:(kt + 1) * 128],
                                    at[:, o:512],
                                    start=(i == 0), stop=(i == last))
                                nc.tensor.matmul(
                                    dn_t[:, o:512], onc[:], at[:, o:512],
                                    start=(i == 0), stop=(i == last))

                            for i in range(len(blocks)):
                                emit_scores(i)
                                if i == 3:
                                    flush()   # prev qc's norm rides here
                                if i >= 2:
                                    emit_avdn(i - 2)
                            flush()           # short block lists
                            for i in range(max(0, len(blocks) - 2),
                                           len(blocks)):
                                emit_avdn(i)
                            pending.append(make_norm(ao_t, dn_t, h, b, qs0))
                flush()

        def phase_d(aot, wo_r):
            with tc.tile_pool(name="pops", bufs=6, space="PSUM") as pops, \
                 tc.tile_pool(name="ostp", bufs=2) as ostp:
                for t in range(T // 128):
                    trows = slice(t * 128, (t + 1) * 128)
                    tb = t // (S // 128)
                    trl = slice((t % (S // 128)) * 128,
                                (t % (S // 128)) * 128 + 128)
                    ostg = ostp.tile([128, H], BF16, tag="ostg", name="ostg")
                    for n in range(H // 512):
                        po = pops.tile([128, 512], F32, tag="po", name="po")
                        for j in range(HQ):
                            nc.tensor.matmul(
                                po[:], aot[j][tb][:, trl],
                                wo_r[:, j, n * 512:(n + 1) * 512],
                                start=(j == 0), stop=(j == HQ - 1))
                        if n % 2 == 0:
                            nc.scalar.copy(
                                ostg[:, n * 512:(n + 1) * 512], po[:])
                        else:
                            nc.vector.tensor_copy(
                                ostg[:, n * 512:(n + 1) * 512], po[:])
                    nc.sync.dma_start(out_d[trows, :], ostg[:, :])

        def body(iv):
            with tc.tile_pool(name="resident", bufs=1) as rp:
                # per-batch tiles so cross-phase deps don't serialize on a
                # whole [128, T] tensor (Tile tracks writers per tile)
                qt = [[rp.tile([128, S], BF16, name=f"qt{m}b{b}",
                               tag=f"qt{m}b{b}") for b in range(B)]
                      for m in range(HQ)]
                ktr = [rp.tile([128, S], BF16, name=f"ktr{b}", tag=f"ktr{b}")
                       for b in range(B)]
                vsb = [rp.tile([128, S], BF16, name=f"vsb{b}", tag=f"vsb{b}")
                       for b in range(B)]
                onc = rp.tile([128, 1], BF16, name="onc")
                onr = rp.tile([2, 128], BF16, name="onr")
                idt = rp.tile([128, 128], BF16, name="idt")
                nc.sync.dma_start(onc[:], onescol_d[:])
                nc.sync.dma_start(onr[:], onesrow_d[:])
                nc.sync.dma_start(idt[:], ident_d[:])

                if phases[0]:
                    with nc.named_scope("phaseB"):
                        phase_b(qt, ktr, vsb, idt)
                with tc.tile_pool(name="aotp", bufs=1) as aotp, \
                     tc.tile_pool(name="wop", bufs=1) as wop:
                    aot = [[aotp.tile([128, S], BF16, name=f"aot{m}b{b}",
                                      tag=f"aot{m}b{b}") for b in range(B)]
                           for m in range(HQ)]
                    wo_r = wop.tile([128, HQ, H], BF16, name="wo_r")
                    # prefetch Wo during phase C so phase D starts immediately
                    nc.sync.dma_start(
                        wo_r[:, :, :],
                        wo_d[:, :].rearrange("(j p) c -> p j c", p=128))
                    if phases[1]:
                        with nc.named_scope("phaseC"):
                            phase_c(qt, ktr, vsb, onc, onr, idt, aot)
                    if phases[2]:
                        with nc.named_scope("phaseD"):
                            phase_d(aot, wo_r)

        if nreps == 1:
            body(0)
        else:
            # large body: arm branch prefetch so the back-edge I$-hits
            with tc.For_i(0, nreps, hint_engines=(
                    mybir.EngineType.PE, mybir.EngineType.Activation,
                    mybir.EngineType.DVE, mybir.EngineType.SP)) as iv:
                body(iv)
    nc.compile()
    return nc


# ---------------------------------------------------------------------------
# Host-side preparation

def _rope_trig(position_ids):
    """cos / sign-swapped-sin in [D, TT] bf16; TT=S if batches share
    positions else T. Returns (cosq, ssq, cosk, ssk, trig_len)."""
    bf = ml_dtypes.bfloat16
    pos = np.asarray(position_ids)
    shared = pos.shape[0] > 1 and np.array_equal(pos[0], pos[1])
    use = pos[:1] if shared else pos
    inv_freq = 1.0 / (ROPE_BASE ** (np.arange(0, D, 2, dtype=np.float32) / D))
    p = use.astype(np.float32)                        # [b', S]
    freqs = p[:, :, None] * inv_freq[None, None, :]   # [b', S, D/2]
    emb = np.concatenate([freqs, freqs], axis=-1)     # [b', S, D]
    cos = np.cos(emb).astype(np.float32)
    sin = np.sin(emb).astype(np.float32)
    cosT = np.concatenate([cos[i].T for i in range(cos.shape[0])], axis=1)
    sinT = np.concatenate([sin[i].T for i in range(sin.shape[0])], axis=1)
    # top half = +sin (multiplies x1 -> rot bottom), bottom half = -sin
    # (multiplies x2 -> rot top); see rope() partition-alignment note.
    ss = sinT.copy()
    ss[D // 2:] *= -1.0
    scale = np.float32(1.0 / np.sqrt(D))
    cosq = (cosT * scale).astype(bf)
    ssq = (ss * scale).astype(bf)
    cosk = cosT.astype(bf)
    ssk = ss.astype(bf)
    return (np.ascontiguousarray(cosq), np.ascontiguousarray(ssq),
            np.ascontiguousarray(cosk), np.ascontiguousarray(ssk),
            cosT.shape[1])


def _classify_mask(attention_mask):
    """Plan over additive [128 k, 512 q] blocks. Returns (plan, uniq_tiles).

    Block kinds: FREE (no mask), DIAG (causal diagonal: cols < off fully
    masked, then the canonical 128x128 triangle, rest free), GEN (arbitrary,
    deduped additive tile), or skipped entirely (fully masked).
    """
    m = np.maximum(np.asarray(attention_mask)[:, 0].astype(np.float32), NEG)
    bf = ml_dtypes.bfloat16
    tri = np.where(np.arange(128)[None, :] >= np.arange(128)[:, None],
                   0.0, NEG).astype(np.float32)
    uniq = {}
    tiles = []
    plan = []
    for b in range(B):
        planb = []
        for qc in range(NQC):
            blocks = []
            qsl = slice(qc * 512, (qc + 1) * 512)
            for kt in range(NKT):
                blk = m[b, qsl, kt * 128:(kt + 1) * 128].T  # [128 k, 512 q]
                if (blk == 0.0).all():
                    blocks.append((kt, FREE, 0))
                    continue
                if (blk <= NEG).all():
                    continue
                o = kt * 128 - qc * 512
                if 0 <= o <= 384:
                    ref = np.full((128, 512), NEG, np.float32)
                    ref[:, o + 128:] = 0.0
                    ref[:, o:o + 128] = tri
                    if (blk == ref).all():
                        blocks.append((kt, DIAG, o))
                        continue
                key = blk.astype(bf).tobytes()
                u = uniq.get(key)
                if u is None:
                    u = len(tiles)
                    uniq[key] = u
                    tiles.append(np.ascontiguousarray(blk.astype(bf)))
                blocks.append((kt, GEN, u))
            planb.append(tuple(blocks))
        plan.append(tuple(planb))
    return tuple(plan), tiles


def _prepare_in_maps(hidden_states, attention_mask, position_ids,
                     Wq, Wk, Wv, Wo):
    bf = ml_dtypes.bfloat16
    X = np.asarray(hidden_states, dtype=np.float32).reshape(T, H)
    XT = np.ascontiguousarray(X.T).astype(bf)  # [H, T]
    cosq, ssq, cosk, ssk, trig_len = _rope_trig(position_ids)
    plan, tiles = _classify_mask(attention_mask)
    n_uniq = len(tiles)
    masks = (np.stack(tiles) if n_uniq
             else np.zeros((1, 128, 512), bf)).astype(bf)
    trimask = np.where(np.arange(128)[None, :] >= np.arange(128)[:, None],
                       0.0, NEG).astype(bf)
    onescol = np.ones((128, 1), np.float32).astype(bf)
    onesrow = np.ones((2, 128), np.float32).astype(bf)
    ident = np.eye(128, dtype=np.float32).astype(bf)
    Wq = np.asarray(Wq, dtype=np.float32)
    Wk = np.asarray(Wk, dtype=np.float32)
    Wv = np.asarray(Wv, dtype=np.float32)
    Wo = np.asarray(Wo, dtype=np.float32)
    in_maps = []
    for c in range(NC):
        wkv = np.concatenate([Wk[:, c * D:(c + 1) * D],
                              Wv[:, c * D:(c + 1) * D]], axis=1)
        in_maps.append({
            "xt": XT,
            "wq": np.ascontiguousarray(
                Wq[:, c * QCOLS:(c + 1) * QCOLS]).astype(bf),
            "wkv": np.ascontiguousarray(wkv).astype(bf),
            "wo": np.ascontiguousarray(
                Wo[c * QCOLS:(c + 1) * QCOLS, :]).astype(bf),
            "cosq": cosq, "ssq": ssq, "cosk": cosk, "ssk": ssk,
            "masks": masks, "trimask": trimask,
            "onescol": onescol, "onesrow": onesrow, "ident": ident,
        })
    return in_maps, (plan, trig_len), n_uniq


# ---------------------------------------------------------------------------
# Execution (modeled on concourse.bass2jax.run_bass_via_pjrt)

_RUNNER_CACHE = {}


class SpmdRunner:
    def __init__(self, nc, n_cores=NC):
        import jax
        from jax.sharding import Mesh, PartitionSpec
        from jax.experimental.shard_map import shard_map
        from concourse.bass2jax import (_bass_exec_p, install_neuronx_cc_hook,
                                        partition_id_tensor)
        self.jax = jax
        install_neuronx_cc_hook()
        self.n_cores = n_cores
        partition_name = (nc.partition_id_tensor.name
                          if nc.partition_id_tensor else None)
        in_names, out_names, out_avals = [], [], []
        for alloc in nc.m.functions[0].allocations:
            if not isinstance(alloc, mybir.MemoryLocationSet):
                continue
            name = alloc.memorylocations[0].name
            if alloc.kind == "ExternalInput":
                in_names.append(name)
            elif alloc.kind == "ExternalOutput":
                out_names.append(name)
                out_avals.append(jax.core.ShapedArray(
                    tuple(alloc.tensor_shape), mybir.dt.np(alloc.dtype)))
        if partition_name is not None and partition_name in in_names:
            in_names.remove(partition_name)
        self.in_names, self.out_names, self.out_avals = \
            in_names, out_names, out_avals
        n_params, n_outs = len(in_names), len(out_avals)
        all_in_names = tuple(in_names + out_names)
        if partition_name is not None:
            all_in_names = all_in_names + (partition_name,)

        def _body(*args):
            operands = list(args)
            if partition_name is not None:
                operands.append(partition_id_tensor())
            outs = _bass_exec_p.bind(
                *operands, out_avals=tuple(out_avals), in_names=all_in_names,
                out_names=tuple(out_names), lowering_input_output_aliases=(),
                sim_require_finite=True, sim_require_nnan=True, nc=nc)
            return tuple(outs)

        devices = jax.devices()[:n_cores]
        mesh = Mesh(np.asarray(devices), ("core",))
        in_specs = (PartitionSpec("core"),) * (n_params + n_outs)
        out_specs = (PartitionSpec("core"),) * n_outs
        self.fn = jax.jit(
            shard_map(_body, mesh=mesh, in_specs=in_specs,
                      out_specs=out_specs, check_rep=False),
            keep_unused=True)

    def prepare(self, in_maps):
        concat_in = [
            np.concatenate([np.asarray(in_maps[c][name])
                            for c in range(self.n_cores)], axis=0)
            for name in self.in_names
        ]
        concat_zeros = [
            np.zeros((self.n_cores * a.shape[0], *a.shape[1:]), a.dtype)
            for a in self.out_avals
        ]
        return [self.jax.device_put(a) for a in concat_in + concat_zeros]

    def run(self, args):
        outs = self.fn(*args)
        self.jax.block_until_ready(outs)
        return outs

    def results(self, outs):
        return [
            {name: np.asarray(outs[i]).reshape(
                self.n_cores, *self.out_avals[i].shape)[c]
             for i, name in enumerate(self.out_names)}
            for c in range(self.n_cores)
        ]


def get_runner(plan_key, n_uniq, nreps=1):
    plan, trig_len = plan_key
    key = (plan, trig_len, n_uniq, nreps)
    r = _RUNNER_CACHE.get(key)
    if r is None:
        nc = _build_program(plan, n_uniq, nreps, trig_len=trig_len)
        r = SpmdRunner(nc)
        _RUNNER_CACHE[key] = r
    return r


def kernel(hidden_states, attention_mask, position_ids, Wq, Wk, Wv, Wo):
    in_maps, plan_key, n_uniq = _prepare_in_maps(
        hidden_states, attention_mask, position_ids, Wq, Wk, Wv, Wo)
    r = get_runner(plan_key, n_uniq, nreps=1)
    outs = r.run(r.prepare(in_maps))
    res = r.results(outs)
    full = res[0]["out"].astype(np.float32).copy()
    for c in range(1, NC):
        full += res[c]["out"]
    return full.reshape(B, S, H)


# revision 5
# speedup vs baseline: 1.1691x; 1.1691x over previous
"""Trainium2 Bass kernel for Llama-style GQA attention (nn_LlamaAttention) — v2.

Shapes (hardcoded): hidden_states [2, 2048, 4096] f32,
attention_mask [2, 1, 2048, 2048] f32, position_ids [2, 2048] i64,
Wq [4096, 4096], Wk/Wv [4096, 1024], Wo [4096, 4096].

Sharding: tensor-parallel over heads across 8 NeuronCores. Core c owns
Q heads 4c..4c+3 and KV head c; host sums the 8 full-shape partials.

v2 changes vs v1 (driven by timeline-sim evidence: DMA-bound phase B,
exp-stalled phase C):
  - Weights SBUF-resident, loaded via a handful of large batched DMAs
    (rearrange APs); xt streamed in quarter-chunk 1MB DMAs. ~70 DMAs/rep
    instead of ~1300 (HWDGE fixed cost 625ns/DMA dominated the old plan).
  - bf16 trig + bf16 PSUM-evict staging; RoPE in 4 DVE ops (3 tile-volumes)
    using a host-precomputed sign-swapped sin layout.
  - Causal diagonal blocks run at reduced width (cols >= 128j only) and the
    triangle mask is ADDED in PSUM via an identity-matmul with a shared
    128x128 additive tile; general masks use the same additive path.
  - Phase C software-pipelined with lookahead 2 (3 score PSUM banks) so the
    ScalarE exp latency hides under the next blocks' score matmuls.
  - Phase D PSUM evictions staged into a [128, 4096] tile, one 2MB DMA per
    token tile; evictions alternate ScalarE/DVE.
"""
import sys
sys.path.insert(0, "/opt/trn_rl_repo")
import numpy as np

import concourse.bass as bass
import concourse.bacc as bacc
import concourse.mybir as mybir
import concourse.tile as tile
import ml_dtypes

F32 = mybir.dt.float32
BF16 = mybir.dt.bfloat16
AF = mybir.ActivationFunctionType
ALU = mybir.AluOpType

H = 4096
NH = 32
NKV = 8
D = 128
B = 2
S = 2048
T = B * S
NC = 8
HQ = NH // NC          # 4 q heads per core
QCOLS = HQ * D         # 512
ROPE_BASE = 10000.0
NQC = S // 512         # 4 q-chunks of 512 per batch
NKT = S // 128         # 16 k-tiles of 128 per batch
NCH = T // 512         # 8 projection chunks
MASK_PRELOAD_MAX = 24
NEG = -100.0           # additive mask value for "fully masked"

# block kinds in plan
FREE, DIAG, GEN = 0, 1, 2


def _build_program(plan, n_uniq, nreps, trig_len=S, phases=(1, 1, 1)):
    """plan[b][qc] = tuple of (kt, kind, arg); arg = col offset for DIAG,
    mask index for GEN, 0 for FREE."""
    nc = bacc.Bacc(None, target_bir_lowering=False)

    xt_d = nc.dram_tensor("xt", [H, T], BF16, kind="ExternalInput")
    wq_d = nc.dram_tensor("wq", [H, QCOLS], BF16, kind="ExternalInput")
    wkv_d = nc.dram_tensor("wkv", [H, 2 * D], BF16, kind="ExternalInput")
    wo_d = nc.dram_tensor("wo", [QCOLS, H], BF16, kind="ExternalInput")
    cosq_d = nc.dram_tensor("cosq", [D, trig_len], BF16, kind="ExternalInput")
    ssq_d = nc.dram_tensor("ssq", [D, trig_len], BF16, kind="ExternalInput")
    cosk_d = nc.dram_tensor("cosk", [D, trig_len], BF16, kind="ExternalInput")
    ssk_d = nc.dram_tensor("ssk", [D, trig_len], BF16, kind="ExternalInput")
    nmask = max(n_uniq, 1)
    masks_d = nc.dram_tensor("masks", [nmask, 128, 512], BF16, kind="ExternalInput")
    trimask_d = nc.dram_tensor("trimask", [128, 128], BF16, kind="ExternalInput")
    onescol_d = nc.dram_tensor("onescol", [128, 1], BF16, kind="ExternalInput")
    onesrow_d = nc.dram_tensor("onesrow", [2, 128], BF16, kind="ExternalInput")
    ident_d = nc.dram_tensor("ident", [128, 128], BF16, kind="ExternalInput")
    out_d = nc.dram_tensor("out", [T, H], BF16, kind="ExternalOutput")

    preload_masks = 0 < n_uniq <= MASK_PRELOAD_MAX

    def trig_off(chunk):
        return (chunk * 512) % trig_len

    with tile.TileContext(nc) as tc:

        def phase_b(qt, ktr, vsb, idt):
            with tc.tile_pool(name="wres", bufs=1) as wres, \
                 tc.tile_pool(name="trig", bufs=1) as trigp, \
                 tc.tile_pool(name="xtp", bufs=3) as xtp, \
                 tc.tile_pool(name="pbps", bufs=1, space="PSUM") as pbps, \
                 tc.tile_pool(name="tps", bufs=2, space="PSUM") as tps, \
                 tc.tile_pool(name="stg", bufs=2) as stg, \
                 tc.tile_pool(name="vbfp", bufs=2) as vbfp, \
                 tc.tile_pool(name="rope", bufs=2) as ropep:
                wq_r = wres.tile([128, 32, QCOLS], BF16, name="wq_r")
                wkv_r = wres.tile([128, 32, 2 * D], BF16, name="wkv_r")
                wq_src = wq_d[:, :].rearrange("(kt p) c -> p kt c", p=128)
                cq = trigp.tile([128, trig_len], BF16, name="cq")
                sq = trigp.tile([128, trig_len], BF16, name="sq")
                ck = trigp.tile([128, trig_len], BF16, name="ck")
                sk = trigp.tile([128, trig_len], BF16, name="sk")

                def load_residents_late():
                    # emitted after chunk-0 xq DMAs: the in-order DMA queue
                    # then delivers (xq0, wq g0) first so the first matmul
                    # starts ~6us in instead of ~25us.
                    for g in range(1, 4):
                        nc.sync.dma_start(wq_r[:, g * 8:(g + 1) * 8, :],
                                          wq_src[:, g * 8:(g + 1) * 8, :])
                    nc.sync.dma_start(
                        wkv_r[:, :, :],
                        wkv_d[:, :].rearrange("(kt p) c -> p kt c", p=128))
                    nc.sync.dma_start(cq[:], cosq_d[:])
                    nc.sync.dma_start(sq[:], ssq_d[:])
                    nc.sync.dma_start(ck[:], cosk_d[:])
                    nc.sync.dma_start(sk[:], ssk_d[:])

                xt_src = xt_d[:, :].rearrange("(kt p) t -> p kt t", p=128)
                pend_tp = []   # deferred V transposes: (vbf_tile, chunk)

                def emit_transposes():
                    while pend_tp:
                        vbf, n0 = pend_tp.pop(0)
                        for j in range(4):
                            ktg = 4 * (n0 % 4) + j
                            tp_t = tps.tile([128, 128], BF16, tag="tp", name="tp_t")
                            nc.tensor.transpose(
                                tp_t[:], vbf[:, j * 128:(j + 1) * 128], idt[:])
                            nc.scalar.copy(
                                vsb[n0 // 4][:, ktg * 128:(ktg + 1) * 128],
                                tp_t[:])

                for n in range(NCH):
                    tok = slice(n * 512, (n + 1) * 512)
                    nb = n // 4
                    tokl = slice((n % 4) * 512, (n % 4) * 512 + 512)
                    to = trig_off(n)
                    # quarter i covers H-row tiles 8i..8i+7 for this token chunk
                    xq = [xtp.tile([128, 8, 512], BF16, tag="xq",
                                   name=f"xq{i}") for i in range(4)]
                    if n == 0:
                        # fine-grained first transfers: k-tiles 0-1 land in
                        # ~1.6us so the first matmuls start almost immediately
                        nc.sync.dma_start(xq[0][:, 0:2, :],
                                          xt_src[:, 0:2, tok])
                        nc.sync.dma_start(wq_r[:, 0:2, :], wq_src[:, 0:2, :])
                        nc.sync.dma_start(xq[0][:, 2:8, :],
                                          xt_src[:, 2:8, tok])
                        nc.sync.dma_start(wq_r[:, 2:8, :], wq_src[:, 2:8, :])
                        for i in range(1, 4):
                            nc.sync.dma_start(xq[i][:, :, :],
                                              xt_src[:, 8 * i:8 * i + 8, tok])
                        load_residents_late()
                    else:
                        for i in range(4):
                            nc.sync.dma_start(xq[i][:, :, :],
                                              xt_src[:, 8 * i:8 * i + 8, tok])
                    ps_q = [pbps.tile([128, 512], F32, tag=f"pq{m}",
                                      name=f"psq{m}") for m in range(HQ)]
                    ps_k = pbps.tile([128, 512], F32, tag="pk", name="psk")
                    ps_v = pbps.tile([128, 512], F32, tag="pv", name="psv")
                    for k in range(32):
                        rhs = xq[k // 8][:, k % 8, :]
                        st = (k == 0)
                        sp = (k == 31)
                        for m in range(HQ):
                            nc.tensor.matmul(
                                ps_q[m][:], wq_r[:, k, m * 128:(m + 1) * 128],
                                rhs, start=st, stop=sp)
                        nc.tensor.matmul(ps_k[:], wkv_r[:, k, 0:D], rhs,
                                         start=st, stop=sp)
                        nc.tensor.matmul(ps_v[:], wkv_r[:, k, D:2 * D], rhs,
                                         start=st, stop=sp)

                    # previous chunk's V transposes ride behind this chunk's MMs
                    emit_transposes()

                    # PSUM -> bf16 staging on ScalarE
                    stq = [stg.tile([128, 512], BF16, tag=f"sq{m}",
                                    name=f"stq{m}") for m in range(HQ)]
                    stk = stg.tile([128, 512], BF16, tag="sk", name="stk")
                    vbf = vbfp.tile([128, 512], BF16, tag="vbf", name="vbf")
                    for m in range(HQ):
                        nc.scalar.copy(stq[m][:], ps_q[m][:])
                    nc.scalar.copy(stk[:], ps_k[:])
                    nc.scalar.copy(vbf[:], ps_v[:])
                    pend_tp.append((vbf, n))

                    # RoPE: out = x*cos + swap(x)*ss   (4 DVE ops)
                    # ss layout: top half = +sin, bottom half = -sin, so each
                    # half-TT reads in0/in1 at the SAME base partition (HW
                    # requires equal base partitions for two SBUF inputs);
                    # the sin halves are identical so this is exact.
                    def rope(dst, x, cos_t, ss_t, nm):
                        rot = ropep.tile([128, 512], BF16, tag="rot",
                                         name=f"rot{nm}")
                        nc.vector.tensor_tensor(
                            rot[0:64, :], x[64:128, :],
                            ss_t[64:128, to:to + 512], ALU.mult)
                        nc.vector.tensor_tensor(
                            rot[64:128, :], x[0:64, :],
                            ss_t[0:64, to:to + 512], ALU.mult)
                        t1 = ropep.tile([128, 512], BF16, tag="t1",
                                        name=f"t1{nm}")
                        nc.vector.tensor_tensor(
                            t1[:], x[:], cos_t[:, to:to + 512], ALU.mult)
                        nc.vector.tensor_tensor(dst, t1[:], rot[:], ALU.add)

                    for m in range(HQ):
                        rope(qt[m][nb][:, tokl], stq[m][:], cq, sq, f"q{m}")
                    rope(ktr[nb][:, tokl], stk[:], ck, sk, "k")

                emit_transposes()

        def phase_c(qt, ktr, vsb, onc, onr, idt, aot):
            with tc.tile_pool(name="scps", bufs=3, space="PSUM") as scps, \
                 tc.tile_pool(name="aops", bufs=2, space="PSUM") as aops, \
                 tc.tile_pool(name="dnps", bufs=2, space="PSUM") as dnps, \
                 tc.tile_pool(name="bcps", bufs=1, space="PSUM") as bcps, \
                 tc.tile_pool(name="atp", bufs=4) as atp, \
                 tc.tile_pool(name="trim", bufs=1) as trimp, \
                 tc.tile_pool(name="mskp", bufs=1 if preload_masks else 4) as mskp, \
                 tc.tile_pool(name="rcp", bufs=2) as rcp:
                tri = trimp.tile([128, 128], BF16, name="tri")
                nc.sync.dma_start(tri[:], trimask_d[:])
                if preload_masks:
                    mres = mskp.tile([128, n_uniq * 512], BF16, name="mres")
                    for u in range(n_uniq):
                        nc.sync.dma_start(mres[:, u * 512:(u + 1) * 512],
                                          masks_d[u])

                # Two-stage deferred normalization: the DVE reciprocal chain
                # is emitted early in the NEXT (b,h,qc) unit (so rhi/rlo are
                # ready), and the PE broadcast matmul + final multiply are
                # emitted at that unit's TAIL — exactly where the PE would
                # otherwise stall waiting on the last block's exp.
                pending = []   # [(dve_stage, pe_stage)]

                def make_norm(ao_t, dn_t, h, bb, qs0):
                    state = {}

                    def dve_stage():
                        rc = rcp.tile([1, 512], F32, tag="rc", name="rc")
                        nc.vector.reciprocal(rc[:], dn_t[:])
                        rhi = rcp.tile([1, 512], BF16, tag="rhi", name="rhi")
                        nc.vector.tensor_copy(rhi[:], rc[:])
                        rlo = rcp.tile([1, 512], BF16, tag="rlo", name="rlo")
                        nc.vector.scalar_tensor_tensor(
                            rlo[:], rc[:], 1.0, rhi[:],
                            ALU.mult, ALU.subtract)
                        state["rhi"], state["rlo"] = rhi, rlo

                    def pe_stage():
                        bc_t = bcps.tile([128, 512], F32, tag="bc",
                                         name="bc_t")
                        nc.tensor.matmul(bc_t[:], onr[0:1, :],
                                         state["rhi"][:],
                                         start=True, stop=False)
                        nc.tensor.matmul(bc_t[:], onr[0:1, :],
                                         state["rlo"][:],
                                         start=False, stop=True)
                        bc_sb = rcp.tile([128, 512], F32, tag="bcsb",
                                         name="bc_sb")
                        nc.vector.tensor_copy(bc_sb[:], bc_t[:])
                        nc.vector.tensor_tensor(
                            aot[h][bb][:, qs0:qs0 + 512], ao_t[:],
                            bc_sb[:], ALU.mult)

                    return (dve_stage, pe_stage)

                def flush_dve():
                    for d, _ in pending:
                        if d is not None:
                            d()
                    pending[:] = [(None, p) for _, p in pending]

                def flush():
                    while pending:
                        d, p = pending.pop(0)
                        if d is not None:
                            d()
                        p()

                # global AV/dn queue: AV+dn of block g are emitted two score-
                # emissions later, ACROSS unit boundaries, so the PE always has
                # score matmuls to chew on while the last exps of a unit land.
                avq = []

                def pump():
                    while len(avq) > 2:
                        avq.pop(0)()

                for b in range(B):
                    for h in range(HQ):
                        for qc in range(NQC):
                            qs0 = qc * 512
                            blocks = plan[b][qc]
                            if len(blocks) == 0:
                                nc.vector.memset(
                                    aot[h][b][:, qs0:qs0 + 512], 0.0)
                                continue
                            last = len(blocks) - 1
                            ao_t = aops.tile([128, 512], F32, tag="ao",
                                             name="ao_t")
                            dn_t = dnps.tile([1, 512], F32, tag="dn",
                                             name="dn_t")
                            sc_tiles = [None] * len(blocks)
                            at_tiles = [None] * len(blocks)
                            offs = [None] * len(blocks)

                            def emit_scores(i):
                                kt, kind, arg = blocks[i]
                                o = arg if kind == DIAG else 0
                                offs[i] = o
                                ks = slice(kt * 128, kt * 128 + 128)
                                sc_t = scps.tile([128, 512], F32, tag="sc",
                                                 name="sc_t")
                                sc_tiles[i] = sc_t
                                if kind == FREE:
                                    nc.tensor.matmul(
                                        sc_t[:], ktr[b][:, ks],
                                        qt[h][b][:, qs0:qs0 + 512],
                                        start=True, stop=True)
                                elif kind == DIAG:
                                    nc.tensor.matmul(
                                        sc_t[:, o:512], ktr[b][:, ks],
                                        qt[h][b][:, qs0 + o:qs0 + 512],
                                        start=True, stop=False)
                                    nc.tensor.matmul(
                                        sc_t[:, o:o + 128], idt[:], tri[:],
                                        start=False, stop=True)
                                else:  # GEN
                                    nc.tensor.matmul(
                                        sc_t[:], ktr[b][:, ks],
                                        qt[h][b][:, qs0:qs0 + 512],
                                        start=True, stop=False)
                                    if preload_masks:
                                        mt = mres[:, arg * 512:(arg + 1) * 512]
                                    else:
                                        mt_t = mskp.tile(
                                            [128, 512], BF16, tag="mst",
                                            name="mst")
                                        nc.sync.dma_start(mt_t[:], masks_d[arg])
                                        mt = mt_t[:]
                                    nc.tensor.matmul(
                                        sc_t[:], idt[:], mt,
                                        start=False, stop=True)
                                at = atp.tile([128, 512], BF16, tag="at",
                                              name="at")
                                at_tiles[i] = at
                                nc.scalar.activation(
                                    at[:, o:512], sc_t[:, o:512], AF.Exp)

                            # deferred AV/dn: bind ALL per-unit state via
                            # default args — the closures outlive this loop
                            # iteration and the loop variables get rebound.
                            def emit_avdn(i, blk=blocks, off=offs,
                                          ats=at_tiles, ao=ao_t, dn=dn_t,
                                          la=last, bb=b):
                                kt, kind, arg = blk[i]
                                o = off[i]
                                at = ats[i]
                                nc.tensor.matmul(
                                    ao[:, o:512],
                                    vsb[bb][:, kt * 128:(kt + 1) * 128],
                                    at[:, o:512],
                                    start=(i == 0), stop=(i == la))
                                nc.tensor.matmul(
                                    dn[:, o:512], onc[:], at[:, o:512],
                                    start=(i == 0), stop=(i == la))

                            nb = len(blocks)
                            for i in range(nb):
                                emit_scores(i)
                                avq.append(
                                    lambda ii=i, f=emit_avdn: f(ii))
                                pump()
                                if i == 1:
                                    flush_dve()   # prev norm's DVE chain
                                if i == 4:
                                    flush()       # prev norm's PE part
                            flush()               # thin units (nb <= 4)
                            pending.append(make_norm(ao_t, dn_t, h, b, qs0))
                while avq:
                    avq.pop(0)()
                flush()

        def phase_d(aot, wo_r):
            with tc.tile_pool(name="pops", bufs=6, space="PSUM") as pops, \
                 tc.tile_pool(name="ostp", bufs=2) as ostp:
                for t in range(T // 128):
                    trows = slice(t * 128, (t + 1) * 128)
                    tb = t // (S // 128)
                    trl = slice((t % (S // 128)) * 128,
                                (t % (S // 128)) * 128 + 128)
                    ostg = ostp.tile([128, H], BF16, tag="ostg", name="ostg")
                    for n in range(H // 512):
                        po = pops.tile([128, 512], F32, tag="po", name="po")
                        for j in range(HQ):
                            nc.tensor.matmul(
                                po[:], aot[j][tb][:, trl],
                                wo_r[:, j, n * 512:(n + 1) * 512],
                                start=(j == 0), stop=(j == HQ - 1))
                        if n % 2 == 0:
                            nc.scalar.copy(
                                ostg[:, n * 512:(n + 1) * 512], po[:])
                        else:
                            nc.vector.tensor_copy(
                                ostg[:, n * 512:(n + 1) * 512], po[:])
                    nc.sync.dma_start(out_d[trows, :], ostg[:, :])

        def body(iv):
            with tc.tile_pool(name="resident", bufs=1) as rp:
                # per-batch tiles so cross-phase deps don't serialize on a
                # whole [128, T] tensor (Tile tracks writers per tile)
                qt = [[rp.tile([128, S], BF16, name=f"qt{m}b{b}",
                               tag=f"qt{m}b{b}") for b in range(B)]
                      for m in range(HQ)]
                ktr = [rp.tile([128, S], BF16, name=f"ktr{b}", tag=f"ktr{b}")
                       for b in range(B)]
                vsb = [rp.tile([128, S], BF16, name=f"vsb{b}", tag=f"vsb{b}")
                       for b in range(B)]
                onc = rp.tile([128, 1], BF16, name="onc")
                onr = rp.tile([2, 128], BF16, name="onr")
                idt = rp.tile([128, 128], BF16, name="idt")
                nc.sync.dma_start(onc[:], onescol_d[:])
                nc.sync.dma_start(onr[:], onesrow_d[:])
                nc.sync.dma_start(idt[:], ident_d[:])

                if phases[0]:
                    with nc.named_scope("phaseB"):
                        phase_b(qt, ktr, vsb, idt)
                with tc.tile_pool(name="aotp", bufs=1) as aotp, \
                     tc.tile_pool(name="wop", bufs=1) as wop:
                    aot = [[aotp.tile([128, S], BF16, name=f"aot{m}b{b}",
                                      tag=f"aot{m}b{b}") for b in range(B)]
                           for m in range(HQ)]
                    wo_r = wop.tile([128, HQ, H], BF16, name="wo_r")
                    # prefetch Wo during phase C so phase D starts immediately
                    nc.sync.dma_start(
                        wo_r[:, :, :],
                        wo_d[:, :].rearrange("(j p) c -> p j c", p=128))
                    if phases[1]:
                        with nc.named_scope("phaseC"):
                            phase_c(qt, ktr, vsb, onc, onr, idt, aot)
                    if phases[2]:
                        with nc.named_scope("phaseD"):
                            phase_d(aot, wo_r)

        if nreps == 1:
            body(0)
        else:
            # large body: arm branch prefetch so the back-edge I$-hits
            with tc.For_i(0, nreps, hint_engines=(
                    mybir.EngineType.PE, mybir.EngineType.Activation,
                    mybir.EngineType.DVE, mybir.EngineType.SP)) as iv:
                body(iv)
    nc.compile()
    return nc


# ---------------------------------------------------------------------------
# Host-side preparation

def _rope_trig(position_ids):
    """cos / sign-swapped-sin in [D, TT] bf16; TT=S if batches share
    positions else T. Returns (cosq, ssq, cosk, ssk, trig_len)."""
    bf = ml_dtypes.bfloat16
    pos = np.asarray(position_ids)
    shared = pos.shape[0] > 1 and np.array_equal(pos[0], pos[1])
    use = pos[:1] if shared else pos
    inv_freq = 1.0 / (ROPE_BASE ** (np.arange(0, D, 2, dtype=np.float32) / D))
    p = use.astype(np.float32)                        # [b', S]
    freqs = p[:, :, None] * inv_freq[None, None, :]   # [b', S, D/2]
    emb = np.concatenate([freqs, freqs], axis=-1)     # [b', S, D]
    cos = np.cos(emb).astype(np.float32)
    sin = np.sin(emb).astype(np.float32)
    cosT = np.concatenate([cos[i].T for i in range(cos.shape[0])], axis=1)
    sinT = np.concatenate([sin[i].T for i in range(sin.shape[0])], axis=1)
    # top half = +sin (multiplies x1 -> rot bottom), bottom half = -sin
    # (multiplies x2 -> rot top); see rope() partition-alignment note.
    ss = sinT.copy()
    ss[D // 2:] *= -1.0
    scale = np.float32(1.0 / np.sqrt(D))
    cosq = (cosT * scale).astype(bf)
    ssq = (ss * scale).astype(bf)
    cosk = cosT.astype(bf)
    ssk = ss.astype(bf)
    return (np.ascontiguousarray(cosq), np.ascontiguousarray(ssq),
            np.ascontiguousarray(cosk), np.ascontiguousarray(ssk),
            cosT.shape[1])


def _classify_mask(attention_mask):
    """Plan over additive [128 k, 512 q] blocks. Returns (plan, uniq_tiles).

    Block kinds: FREE (no mask), DIAG (causal diagonal: cols < off fully
    masked, then the canonical 128x128 triangle, rest free), GEN (arbitrary,
    deduped additive tile), or skipped entirely (fully masked).
    """
    m = np.maximum(np.asarray(attention_mask)[:, 0].astype(np.float32), NEG)
    bf = ml_dtypes.bfloat16
    tri = np.where(np.arange(128)[None, :] >= np.arange(128)[:, None],
                   0.0, NEG).astype(np.float32)
    uniq = {}
    tiles = []
    plan = []
    for b in range(B):
        planb = []
        for qc in range(NQC):
            blocks = []
            qsl = slice(qc * 512, (qc + 1) * 512)
            for kt in range(NKT):
                blk = m[b, qsl, kt * 128:(kt + 1) * 128].T  # [128 k, 512 q]
                if (blk == 0.0).all():
                    blocks.append((kt, FREE, 0))
                    continue
                if (blk <= NEG).all():
                    continue
                o = kt * 128 - qc * 512
                if 0 <= o <= 384:
                    ref = np.full((128, 512), NEG, np.float32)
                    ref[:, o + 128:] = 0.0
                    ref[:, o:o + 128] = tri
                    if (blk == ref).all():
                        blocks.append((kt, DIAG, o))
                        continue
                key = blk.astype(bf).tobytes()
                u = uniq.get(key)
                if u is None:
                    u = len(tiles)
                    uniq[key] = u
                    tiles.append(np.ascontiguousarray(blk.astype(bf)))
                blocks.append((kt, GEN, u))
            planb.append(tuple(blocks))
        plan.append(tuple(planb))
    return tuple(plan), tiles


def _prepare_in_maps(hidden_states, attention_mask, position_ids,
                     Wq, Wk, Wv, Wo):
    bf = ml_dtypes.bfloat16
    X = np.asarray(hidden_states, dtype=np.float32).reshape(T, H)
    XT = np.ascontiguousarray(X.T).astype(bf)  # [H, T]
    cosq, ssq, cosk, ssk, trig_len = _rope_trig(position_ids)
    plan, tiles = _classify_mask(attention_mask)
    n_uniq = len(tiles)
    masks = (np.stack(tiles) if n_uniq
             else np.zeros((1, 128, 512), bf)).astype(bf)
    trimask = np.where(np.arange(128)[None, :] >= np.arange(128)[:, None],
                       0.0, NEG).astype(bf)
    onescol = np.ones((128, 1), np.float32).astype(bf)
    onesrow = np.ones((2, 128), np.float32).astype(bf)
    ident = np.eye(128, dtype=np.float32).astype(bf)
    Wq = np.asarray(Wq, dtype=np.float32)
    Wk = np.asarray(Wk, dtype=np.float32)
    Wv = np.asarray(Wv, dtype=np.float32)
    Wo = np.asarray(Wo, dtype=np.float32)
    in_maps = []
    for c in range(NC):
        wkv = np.concatenate([Wk[:, c * D:(c + 1) * D],
                              Wv[:, c * D:(c + 1) * D]], axis=1)
        in_maps.append({
            "xt": XT,
            "wq": np.ascontiguousarray(
                Wq[:, c * QCOLS:(c + 1) * QCOLS]).astype(bf),
            "wkv": np.ascontiguousarray(wkv).astype(bf),
            "wo": np.ascontiguousarray(
                Wo[c * QCOLS:(c + 1) * QCOLS, :]).astype(bf),
            "cosq": cosq, "ssq": ssq, "cosk": cosk, "ssk": ssk,
            "masks": masks, "trimask": trimask,
            "onescol": onescol, "onesrow": onesrow, "ident": ident,
        })
    return in_maps, (plan, trig_len), n_uniq


# ---------------------------------------------------------------------------
# Execution (modeled on concourse.bass2jax.run_bass_via_pjrt)

_RUNNER_CACHE = {}


class SpmdRunner:
    def __init__(self, nc, n_cores=NC):
        import jax
        from jax.sharding import Mesh, PartitionSpec
        from jax.experimental.shard_map import shard_map
        from concourse.bass2jax import (_bass_exec_p, install_neuronx_cc_hook,
                                        partition_id_tensor)
        self.jax = jax
        install_neuronx_cc_hook()
        self.n_cores = n_cores
        partition_name = (nc.partition_id_tensor.name
                          if nc.partition_id_tensor else None)
        in_names, out_names, out_avals = [], [], []
        for alloc in nc.m.functions[0].allocations:
            if not isinstance(alloc, mybir.MemoryLocationSet):
                continue
            name = alloc.memorylocations[0].name
            if alloc.kind == "ExternalInput":
                in_names.append(name)
            elif alloc.kind == "ExternalOutput":
                out_names.append(name)
                out_avals.append(jax.core.ShapedArray(
                    tuple(alloc.tensor_shape), mybir.dt.np(alloc.dtype)))
        if partition_name is not None and partition_name in in_names:
            in_names.remove(partition_name)
        self.in_names, self.out_names, self.out_avals = \
            in_names, out_names, out_avals
        n_params, n_outs = len(in_names), len(out_avals)
        all_in_names = tuple(in_names + out_names)
        if partition_name is not None:
            all_in_names = all_in_names + (partition_name,)

        def _body(*args):
            operands = list(args)
            if partition_name is not None:
                operands.append(partition_id_tensor())
            outs = _bass_exec_p.bind(
                *operands, out_avals=tuple(out_avals), in_names=all_in_names,
                out_names=tuple(out_names), lowering_input_output_aliases=(),
                sim_require_finite=True, sim_require_nnan=True, nc=nc)
            return tuple(outs)

        devices = jax.devices()[:n_cores]
        mesh = Mesh(np.asarray(devices), ("core",))
        in_specs = (PartitionSpec("core"),) * (n_params + n_outs)
        out_specs = (PartitionSpec("core"),) * n_outs
        self.fn = jax.jit(
            shard_map(_body, mesh=mesh, in_specs=in_specs,
                      out_specs=out_specs, check_rep=False),
            keep_unused=True)

    def prepare(self, in_maps):
        concat_in = [
            np.concatenate([np.asarray(in_maps[c][name])
                            for c in range(self.n_cores)], axis=0)
            for name in self.in_names
        ]
        concat_zeros = [
            np.zeros((self.n_cores * a.shape[0], *a.shape[1:]), a.dtype)
            for a in self.out_avals
        ]
        return [self.jax.device_put(a) for a in concat_in + concat_zeros]

    def run(self, args):
        outs = self.fn(*args)
        self.jax.block_until_ready(outs)
        return outs

    def results(self, outs):
        return [
            {name: np.asarray(outs[i]).reshape(
                self.n_cores, *self.out_avals[i].shape)[c]
             for i, name in enumerate(self.out_names)}
            for c in range(self.n_cores)
        ]


def get_runner(plan_key, n_uniq, nreps=1):
    plan, trig_len = plan_key
    key = (plan, trig_len, n_uniq, nreps)
    r = _RUNNER_CACHE.get(key)
    if r is None:
        nc = _build_program(plan, n_uniq, nreps, trig_len=trig_len)
        r = SpmdRunner(nc)
        _RUNNER_CACHE[key] = r
    return r


def kernel(hidden_states, attention_mask, position_ids, Wq, Wk, Wv, Wo):
    in_maps, plan_key, n_uniq = _prepare_in_maps(
        hidden_states, attention_mask, position_ids, Wq, Wk, Wv, Wo)
    r = get_runner(plan_key, n_uniq, nreps=1)
    outs = r.run(r.prepare(in_maps))
    res = r.results(outs)
    full = res[0]["out"].astype(np.float32).copy()
    for c in range(1, NC):
        full += res[c]["out"]
    return full.reshape(B, S, H)


# revision 6
# speedup vs baseline: 1.2690x; 1.0855x over previous
"""Trainium2 Bass kernel for Llama-style GQA attention (nn_LlamaAttention) — v2.

Shapes (hardcoded): hidden_states [2, 2048, 4096] f32,
attention_mask [2, 1, 2048, 2048] f32, position_ids [2, 2048] i64,
Wq [4096, 4096], Wk/Wv [4096, 1024], Wo [4096, 4096].

Sharding: tensor-parallel over heads across 8 NeuronCores. Core c owns
Q heads 4c..4c+3 and KV head c; host sums the 8 full-shape partials.

v2 changes vs v1 (driven by timeline-sim evidence: DMA-bound phase B,
exp-stalled phase C):
  - Weights SBUF-resident, loaded via a handful of large batched DMAs
    (rearrange APs); xt streamed in quarter-chunk 1MB DMAs. ~70 DMAs/rep
    instead of ~1300 (HWDGE fixed cost 625ns/DMA dominated the old plan).
  - bf16 trig + bf16 PSUM-evict staging; RoPE in 4 DVE ops (3 tile-volumes)
    using a host-precomputed sign-swapped sin layout.
  - Causal diagonal blocks run at reduced width (cols >= 128j only) and the
    triangle mask is ADDED in PSUM via an identity-matmul with a shared
    128x128 additive tile; general masks use the same additive path.
  - Phase C software-pipelined with lookahead 2 (3 score PSUM banks) so the
    ScalarE exp latency hides under the next blocks' score matmuls.
  - Phase D PSUM evictions staged into a [128, 4096] tile, one 2MB DMA per
    token tile; evictions alternate ScalarE/DVE.
"""
import sys
sys.path.insert(0, "/opt/trn_rl_repo")
import numpy as np

import concourse.bass as bass
import concourse.bacc as bacc
import concourse.mybir as mybir
import concourse.tile as tile
import ml_dtypes

F32 = mybir.dt.float32
BF16 = mybir.dt.bfloat16
AF = mybir.ActivationFunctionType
ALU = mybir.AluOpType

H = 4096
NH = 32
NKV = 8
D = 128
B = 2
S = 2048
T = B * S
NC = 8
HQ = NH // NC          # 4 q heads per core
QCOLS = HQ * D         # 512
ROPE_BASE = 10000.0
NQC = S // 512         # 4 q-chunks of 512 per batch
NKT = S // 128         # 16 k-tiles of 128 per batch
NCH = T // 512         # 8 projection chunks
MASK_PRELOAD_MAX = 24
NEG = -100.0           # additive mask value for "fully masked"

# block kinds in plan
FREE, DIAG, GEN = 0, 1, 2


def _build_program(plan, n_uniq, nreps, trig_len=S, phases=(1, 1, 1)):
    """plan[b][qc] = tuple of (kt, kind, arg); arg = col offset for DIAG,
    mask index for GEN, 0 for FREE."""
    nc = bacc.Bacc(None, target_bir_lowering=False)

    xt_d = nc.dram_tensor("xt", [H, T], BF16, kind="ExternalInput")
    wq_d = nc.dram_tensor("wq", [H, QCOLS], BF16, kind="ExternalInput")
    wkv_d = nc.dram_tensor("wkv", [H, 2 * D], BF16, kind="ExternalInput")
    wo_d = nc.dram_tensor("wo", [QCOLS, H], BF16, kind="ExternalInput")
    cosq_d = nc.dram_tensor("cosq", [D, trig_len], BF16, kind="ExternalInput")
    ssq_d = nc.dram_tensor("ssq", [D, trig_len], BF16, kind="ExternalInput")
    cosk_d = nc.dram_tensor("cosk", [D, trig_len], BF16, kind="ExternalInput")
    ssk_d = nc.dram_tensor("ssk", [D, trig_len], BF16, kind="ExternalInput")
    nmask = max(n_uniq, 1)
    masks_d = nc.dram_tensor("masks", [nmask, 128, 512], BF16, kind="ExternalInput")
    trimask_d = nc.dram_tensor("trimask", [128, 128], BF16, kind="ExternalInput")
    onescol_d = nc.dram_tensor("onescol", [128, 1], BF16, kind="ExternalInput")
    onesrow_d = nc.dram_tensor("onesrow", [2, 128], BF16, kind="ExternalInput")
    ident_d = nc.dram_tensor("ident", [128, 128], BF16, kind="ExternalInput")
    out_d = nc.dram_tensor("out", [T, H], BF16, kind="ExternalOutput")

    preload_masks = 0 < n_uniq <= MASK_PRELOAD_MAX

    def trig_off(chunk):
        return (chunk * 512) % trig_len

    with tile.TileContext(nc) as tc:

        def phase_b(qt, ktr, vsb, onc, onr, idt):
            with tc.tile_pool(name="wres", bufs=1) as wres, \
                 tc.tile_pool(name="trig", bufs=1) as trigp, \
                 tc.tile_pool(name="xtp", bufs=3) as xtp, \
                 tc.tile_pool(name="pbps", bufs=1, space="PSUM") as pbps, \
                 tc.tile_pool(name="tps", bufs=2, space="PSUM") as tps, \
                 tc.tile_pool(name="stg", bufs=2) as stg, \
                 tc.tile_pool(name="vbfp", bufs=2) as vbfp, \
                 tc.tile_pool(name="rope", bufs=2) as ropep:
                wq_r = wres.tile([128, 32, QCOLS], BF16, name="wq_r")
                wkv_r = wres.tile([128, 32, 2 * D], BF16, name="wkv_r")
                wq_src = wq_d[:, :].rearrange("(kt p) c -> p kt c", p=128)
                cq = trigp.tile([128, trig_len], BF16, name="cq")
                sq = trigp.tile([128, trig_len], BF16, name="sq")
                ck = trigp.tile([128, trig_len], BF16, name="ck")
                sk = trigp.tile([128, trig_len], BF16, name="sk")

                def load_trig_etc(onc, onr, idt):
                    nc.sync.dma_start(
                        wkv_r[:, :, :],
                        wkv_d[:, :].rearrange("(kt p) c -> p kt c", p=128))
                    nc.sync.dma_start(cq[:], cosq_d[:])
                    nc.sync.dma_start(sq[:], ssq_d[:])
                    nc.sync.dma_start(ck[:], cosk_d[:])
                    nc.sync.dma_start(sk[:], ssk_d[:])
                    nc.sync.dma_start(onc[:], onescol_d[:])
                    nc.sync.dma_start(onr[:], onesrow_d[:])
                    nc.sync.dma_start(idt[:], ident_d[:])

                xt_src = xt_d[:, :].rearrange("(kt p) t -> p kt t", p=128)
                pend_tp = []   # deferred V transposes: (vbf_tile, chunk)

                def emit_transposes():
                    while pend_tp:
                        vbf, n0 = pend_tp.pop(0)
                        for j in range(4):
                            ktg = 4 * (n0 % 4) + j
                            tp_t = tps.tile([128, 128], BF16, tag="tp", name="tp_t")
                            nc.tensor.transpose(
                                tp_t[:], vbf[:, j * 128:(j + 1) * 128], idt[:])
                            nc.scalar.copy(
                                vsb[n0 // 4][:, ktg * 128:(ktg + 1) * 128],
                                tp_t[:])

                for n in range(NCH):
                    tok = slice(n * 512, (n + 1) * 512)
                    nb = n // 4
                    tokl = slice((n % 4) * 512, (n % 4) * 512 + 512)
                    to = trig_off(n)
                    # quarter i covers H-row tiles 8i..8i+7 for this token chunk
                    xq = [xtp.tile([128, 8, 512], BF16, tag="xq",
                                   name=f"xq{i}") for i in range(4)]
                    if n == 0:
                        # fine-grained first transfers: k-tiles 0-1 land in
                        # ~1.6us so the first matmuls start almost immediately;
                        # each wq group rides right behind its xq quarter so
                        # the in-order DMA queue delivers weights just in time.
                        nc.sync.dma_start(xq[0][:, 0:2, :],
                                          xt_src[:, 0:2, tok])
                        nc.sync.dma_start(wq_r[:, 0:2, :], wq_src[:, 0:2, :])
                        nc.sync.dma_start(xq[0][:, 2:8, :],
                                          xt_src[:, 2:8, tok])
                        nc.sync.dma_start(wq_r[:, 2:8, :], wq_src[:, 2:8, :])
                        for i in range(1, 4):
                            nc.sync.dma_start(xq[i][:, :, :],
                                              xt_src[:, 8 * i:8 * i + 8, tok])
                            nc.sync.dma_start(
                                wq_r[:, 8 * i:8 * i + 8, :],
                                wq_src[:, 8 * i:8 * i + 8, :])
                        load_trig_etc(onc, onr, idt)
                    else:
                        for i in range(4):
                            nc.sync.dma_start(xq[i][:, :, :],
                                              xt_src[:, 8 * i:8 * i + 8, tok])
                    ps_q = [pbps.tile([128, 512], F32, tag=f"pq{m}",
                                      name=f"psq{m}") for m in range(HQ)]
                    ps_k = pbps.tile([128, 512], F32, tag="pk", name="psk")
                    ps_v = pbps.tile([128, 512], F32, tag="pv", name="psv")
                    for k in range(32):
                        rhs = xq[k // 8][:, k % 8, :]
                        st = (k == 0)
                        sp = (k == 31)
                        for m in range(HQ):
                            nc.tensor.matmul(
                                ps_q[m][:], wq_r[:, k, m * 128:(m + 1) * 128],
                                rhs, start=st, stop=sp)
                        nc.tensor.matmul(ps_k[:], wkv_r[:, k, 0:D], rhs,
                                         start=st, stop=sp)
                        nc.tensor.matmul(ps_v[:], wkv_r[:, k, D:2 * D], rhs,
                                         start=st, stop=sp)

                    # previous chunk's V transposes ride behind this chunk's MMs
                    emit_transposes()

                    # PSUM -> bf16 staging on ScalarE
                    stq = [stg.tile([128, 512], BF16, tag=f"sq{m}",
                                    name=f"stq{m}") for m in range(HQ)]
                    stk = stg.tile([128, 512], BF16, tag="sk", name="stk")
                    vbf = vbfp.tile([128, 512], BF16, tag="vbf", name="vbf")
                    for m in range(HQ):
                        nc.scalar.copy(stq[m][:], ps_q[m][:])
                    nc.scalar.copy(stk[:], ps_k[:])
                    nc.scalar.copy(vbf[:], ps_v[:])
                    pend_tp.append((vbf, n))

                    # RoPE: out = x*cos + swap(x)*ss   (4 DVE ops)
                    # ss layout: top half = +sin, bottom half = -sin, so each
                    # half-TT reads in0/in1 at the SAME base partition (HW
                    # requires equal base partitions for two SBUF inputs);
                    # the sin halves are identical so this is exact.
                    def rope(dst, x, cos_t, ss_t, nm):
                        rot = ropep.tile([128, 512], BF16, tag="rot",
                                         name=f"rot{nm}")
                        nc.vector.tensor_tensor(
                            rot[0:64, :], x[64:128, :],
                            ss_t[64:128, to:to + 512], ALU.mult)
                        nc.vector.tensor_tensor(
                            rot[64:128, :], x[0:64, :],
                            ss_t[0:64, to:to + 512], ALU.mult)
                        t1 = ropep.tile([128, 512], BF16, tag="t1",
                                        name=f"t1{nm}")
                        nc.vector.tensor_tensor(
                            t1[:], x[:], cos_t[:, to:to + 512], ALU.mult)
                        nc.vector.tensor_tensor(dst, t1[:], rot[:], ALU.add)

                    for m in range(HQ):
                        rope(qt[m][nb][:, tokl], stq[m][:], cq, sq, f"q{m}")
                    rope(ktr[nb][:, tokl], stk[:], ck, sk, "k")

                emit_transposes()

        def phase_c(qt, ktr, vsb, onc, onr, idt, aot):
            with tc.tile_pool(name="scps", bufs=3, space="PSUM") as scps, \
                 tc.tile_pool(name="aops", bufs=2, space="PSUM") as aops, \
                 tc.tile_pool(name="dnps", bufs=2, space="PSUM") as dnps, \
                 tc.tile_pool(name="bcps", bufs=1, space="PSUM") as bcps, \
                 tc.tile_pool(name="atp", bufs=4) as atp, \
                 tc.tile_pool(name="trim", bufs=1) as trimp, \
                 tc.tile_pool(name="mskp", bufs=1 if preload_masks else 4) as mskp, \
                 tc.tile_pool(name="rcp", bufs=2) as rcp:
                tri = trimp.tile([128, 128], BF16, name="tri")
                nc.sync.dma_start(tri[:], trimask_d[:])
                if preload_masks:
                    mres = mskp.tile([128, n_uniq * 512], BF16, name="mres")
                    for u in range(n_uniq):
                        nc.sync.dma_start(mres[:, u * 512:(u + 1) * 512],
                                          masks_d[u])

                # Two-stage deferred normalization: the DVE reciprocal chain
                # is emitted early in the NEXT (b,h,qc) unit (so rhi/rlo are
                # ready), and the PE broadcast matmul + final multiply are
                # emitted at that unit's TAIL — exactly where the PE would
                # otherwise stall waiting on the last block's exp.
                pending = []   # [(dve_stage, pe_stage)]

                def make_norm(ao_t, dn_t, h, bb, qs0):
                    state = {}

                    def dve_stage():
                        rc = rcp.tile([1, 512], F32, tag="rc", name="rc")
                        nc.vector.reciprocal(rc[:], dn_t[:])
                        rhi = rcp.tile([1, 512], BF16, tag="rhi", name="rhi")
                        nc.vector.tensor_copy(rhi[:], rc[:])
                        rlo = rcp.tile([1, 512], BF16, tag="rlo", name="rlo")
                        nc.vector.scalar_tensor_tensor(
                            rlo[:], rc[:], 1.0, rhi[:],
                            ALU.mult, ALU.subtract)
                        state["rhi"], state["rlo"] = rhi, rlo

                    def pe_stage():
                        bc_t = bcps.tile([128, 512], F32, tag="bc",
                                         name="bc_t")
                        nc.tensor.matmul(bc_t[:], onr[0:1, :],
                                         state["rhi"][:],
                                         start=True, stop=False)
                        nc.tensor.matmul(bc_t[:], onr[0:1, :],
                                         state["rlo"][:],
                                         start=False, stop=True)
                        bc_sb = rcp.tile([128, 512], F32, tag="bcsb",
                                         name="bc_sb")
                        nc.vector.tensor_copy(bc_sb[:], bc_t[:])
                        nc.vector.tensor_tensor(
                            aot[h][bb][:, qs0:qs0 + 512], ao_t[:],
                            bc_sb[:], ALU.mult)

                    return (dve_stage, pe_stage)

                def flush_dve():
                    for d, _ in pending:
                        if d is not None:
                            d()
                    pending[:] = [(None, p) for _, p in pending]

                def flush():
                    while pending:
                        d, p = pending.pop(0)
                        if d is not None:
                            d()
                        p()

                # global AV/dn queue: AV+dn of block g are emitted two score-
                # emissions later, ACROSS unit boundaries, so the PE always has
                # score matmuls to chew on while the last exps of a unit land.
                avq = []

                def pump():
                    while len(avq) > 2:
                        avq.pop(0)()

                for b in range(B):
                    for h in range(HQ):
                        for qc in range(NQC):
                            qs0 = qc * 512
                            blocks = plan[b][qc]
                            if len(blocks) == 0:
                                nc.vector.memset(
                                    aot[h][b][:, qs0:qs0 + 512], 0.0)
                                continue
                            last = len(blocks) - 1
                            ao_t = aops.tile([128, 512], F32, tag="ao",
                                             name="ao_t")
                            dn_t = dnps.tile([1, 512], F32, tag="dn",
                                             name="dn_t")
                            sc_tiles = [None] * len(blocks)
                            at_tiles = [None] * len(blocks)
                            offs = [None] * len(blocks)

                            def emit_scores(i):
                                kt, kind, arg = blocks[i]
                                o = arg if kind == DIAG else 0
                                offs[i] = o
                                ks = slice(kt * 128, kt * 128 + 128)
                                sc_t = scps.tile([128, 512], F32, tag="sc",
                                                 name="sc_t")
                                sc_tiles[i] = sc_t
                                if kind == FREE:
                                    nc.tensor.matmul(
                                        sc_t[:], ktr[b][:, ks],
                                        qt[h][b][:, qs0:qs0 + 512],
                                        start=True, stop=True)
                                elif kind == DIAG:
                                    nc.tensor.matmul(
                                        sc_t[:, o:512], ktr[b][:, ks],
                                        qt[h][b][:, qs0 + o:qs0 + 512],
                                        start=True, stop=False)
                                    nc.tensor.matmul(
                                        sc_t[:, o:o + 128], idt[:], tri[:],
                                        start=False, stop=True)
                                else:  # GEN
                                    nc.tensor.matmul(
                                        sc_t[:], ktr[b][:, ks],
                                        qt[h][b][:, qs0:qs0 + 512],
                                        start=True, stop=False)
                                    if preload_masks:
                                        mt = mres[:, arg * 512:(arg + 1) * 512]
                                    else:
                                        mt_t = mskp.tile(
                                            [128, 512], BF16, tag="mst",
                                            name="mst")
                                        nc.sync.dma_start(mt_t[:], masks_d[arg])
                                        mt = mt_t[:]
                                    nc.tensor.matmul(
                                        sc_t[:], idt[:], mt,
                                        start=False, stop=True)
                                at = atp.tile([128, 512], BF16, tag="at",
                                              name="at")
                                at_tiles[i] = at
                                nc.scalar.activation(
                                    at[:, o:512], sc_t[:, o:512], AF.Exp)

                            # deferred AV/dn: bind ALL per-unit state via
                            # default args — the closures outlive this loop
                            # iteration and the loop variables get rebound.
                            def emit_avdn(i, blk=blocks, off=offs,
                                          ats=at_tiles, ao=ao_t, dn=dn_t,
                                          la=last, bb=b):
                                kt, kind, arg = blk[i]
                                o = off[i]
                                at = ats[i]
                                nc.tensor.matmul(
                                    ao[:, o:512],
                                    vsb[bb][:, kt * 128:(kt + 1) * 128],
                                    at[:, o:512],
                                    start=(i == 0), stop=(i == la))
                                nc.tensor.matmul(
                                    dn[:, o:512], onc[:], at[:, o:512],
                                    start=(i == 0), stop=(i == la))

                            nb = len(blocks)
                            for i in range(nb):
                                emit_scores(i)
                                avq.append(
                                    lambda ii=i, f=emit_avdn: f(ii))
                                pump()
                                if i == 1:
                                    flush_dve()   # prev norm's DVE chain
                                if i == 4:
                                    flush()       # prev norm's PE part
                            flush()               # thin units (nb <= 4)
                            pending.append(make_norm(ao_t, dn_t, h, b, qs0))
                while avq:
                    avq.pop(0)()
                flush()

        def phase_d(aot, wo_r):
            with tc.tile_pool(name="pops", bufs=6, space="PSUM") as pops, \
                 tc.tile_pool(name="ostp", bufs=2) as ostp:
                for t in range(T // 128):
                    trows = slice(t * 128, (t + 1) * 128)
                    tb = t // (S // 128)
                    trl = slice((t % (S // 128)) * 128,
                                (t % (S // 128)) * 128 + 128)
                    ostg = ostp.tile([128, H], BF16, tag="ostg", name="ostg")
                    for n in range(H // 512):
                        po = pops.tile([128, 512], F32, tag="po", name="po")
                        for j in range(HQ):
                            nc.tensor.matmul(
                                po[:], aot[j][tb][:, trl],
                                wo_r[:, j, n * 512:(n + 1) * 512],
                                start=(j == 0), stop=(j == HQ - 1))
                        if n % 2 == 0:
                            nc.scalar.copy(
                                ostg[:, n * 512:(n + 1) * 512], po[:])
                        else:
                            nc.vector.tensor_copy(
                                ostg[:, n * 512:(n + 1) * 512], po[:])
                    nc.sync.dma_start(out_d[trows, :], ostg[:, :])

        def body(iv):
            with tc.tile_pool(name="resident", bufs=1) as rp:
                # per-batch tiles so cross-phase deps don't serialize on a
                # whole [128, T] tensor (Tile tracks writers per tile)
                qt = [[rp.tile([128, S], BF16, name=f"qt{m}b{b}",
                               tag=f"qt{m}b{b}") for b in range(B)]
                      for m in range(HQ)]
                ktr = [rp.tile([128, S], BF16, name=f"ktr{b}", tag=f"ktr{b}")
                       for b in range(B)]
                vsb = [rp.tile([128, S], BF16, name=f"vsb{b}", tag=f"vsb{b}")
                       for b in range(B)]
                onc = rp.tile([128, 1], BF16, name="onc")
                onr = rp.tile([2, 128], BF16, name="onr")
                idt = rp.tile([128, 128], BF16, name="idt")

                if phases[0]:
                    with nc.named_scope("phaseB"):
                        phase_b(qt, ktr, vsb, onc, onr, idt)
                else:
                    nc.sync.dma_start(onc[:], onescol_d[:])
                    nc.sync.dma_start(onr[:], onesrow_d[:])
                    nc.sync.dma_start(idt[:], ident_d[:])
                with tc.tile_pool(name="aotp", bufs=1) as aotp, \
                     tc.tile_pool(name="wop", bufs=1) as wop:
                    aot = [[aotp.tile([128, S], BF16, name=f"aot{m}b{b}",
                                      tag=f"aot{m}b{b}") for b in range(B)]
                           for m in range(HQ)]
                    wo_r = wop.tile([128, HQ, H], BF16, name="wo_r")
                    # prefetch Wo during phase C so phase D starts immediately
                    nc.sync.dma_start(
                        wo_r[:, :, :],
                        wo_d[:, :].rearrange("(j p) c -> p j c", p=128))
                    if phases[1]:
                        with nc.named_scope("phaseC"):
                            phase_c(qt, ktr, vsb, onc, onr, idt, aot)
                    if phases[2]:
                        with nc.named_scope("phaseD"):
                            phase_d(aot, wo_r)

        if nreps == 1:
            body(0)
        else:
            # large body: arm branch prefetch so the back-edge I$-hits
            with tc.For_i(0, nreps, hint_engines=(
                    mybir.EngineType.PE, mybir.EngineType.Activation,
                    mybir.EngineType.DVE, mybir.EngineType.SP)) as iv:
                body(iv)
    nc.compile()
    return nc


# ---------------------------------------------------------------------------
# Host-side preparation

def _rope_trig(position_ids):
    """cos / sign-swapped-sin in [D, TT] bf16; TT=S if batches share
    positions else T. Returns (cosq, ssq, cosk, ssk, trig_len)."""
    bf = ml_dtypes.bfloat16
    pos = np.asarray(position_ids)
    shared = pos.shape[0] > 1 and np.array_equal(pos[0], pos[1])
    use = pos[:1] if shared else pos
    inv_freq = 1.0 / (ROPE_BASE ** (np.arange(0, D, 2, dtype=np.float32) / D))
    p = use.astype(np.float32)                        # [b', S]
    freqs = p[:, :, None] * inv_freq[None, None, :]   # [b', S, D/2]
    emb = np.concatenate([freqs, freqs], axis=-1)     # [b', S, D]
    cos = np.cos(emb).astype(np.float32)
    sin = np.sin(emb).astype(np.float32)
    cosT = np.concatenate([cos[i].T for i in range(cos.shape[0])], axis=1)
    sinT = np.concatenate([sin[i].T for i in range(sin.shape[0])], axis=1)
    # top half = +sin (multiplies x1 -> rot bottom), bottom half = -sin
    # (multiplies x2 -> rot top); see rope() partition-alignment note.
    ss = sinT.copy()
    ss[D // 2:] *= -1.0
    scale = np.float32(1.0 / np.sqrt(D))
    cosq = (cosT * scale).astype(bf)
    ssq = (ss * scale).astype(bf)
    cosk = cosT.astype(bf)
    ssk = ss.astype(bf)
    return (np.ascontiguousarray(cosq), np.ascontiguousarray(ssq),
            np.ascontiguousarray(cosk), np.ascontiguousarray(ssk),
            cosT.shape[1])


def _classify_mask(attention_mask):
    """Plan over additive [128 k, 512 q] blocks. Returns (plan, uniq_tiles).

    Block kinds: FREE (no mask), DIAG (causal diagonal: cols < off fully
    masked, then the canonical 128x128 triangle, rest free), GEN (arbitrary,
    deduped additive tile), or skipped entirely (fully masked).
    """
    m = np.maximum(np.asarray(attention_mask)[:, 0].astype(np.float32), NEG)
    bf = ml_dtypes.bfloat16
    tri = np.where(np.arange(128)[None, :] >= np.arange(128)[:, None],
                   0.0, NEG).astype(np.float32)
    uniq = {}
    tiles = []
    plan = []
    for b in range(B):
        planb = []
        for qc in range(NQC):
            blocks = []
            qsl = slice(qc * 512, (qc + 1) * 512)
            for kt in range(NKT):
                blk = m[b, qsl, kt * 128:(kt + 1) * 128].T  # [128 k, 512 q]
                if (blk == 0.0).all():
                    blocks.append((kt, FREE, 0))
                    continue
                if (blk <= NEG).all():
                    continue
                o = kt * 128 - qc * 512
                if 0 <= o <= 384:
                    ref = np.full((128, 512), NEG, np.float32)
                    ref[:, o + 128:] = 0.0
                    ref[:, o:o + 128] = tri
                    if (blk == ref).all():
                        blocks.append((kt, DIAG, o))
                        continue
                key = blk.astype(bf).tobytes()
                u = uniq.get(key)
                if u is None:
                    u = len(tiles)
                    uniq[key] = u
                    tiles.append(np.ascontiguousarray(blk.astype(bf)))
                blocks.append((kt, GEN, u))
            planb.append(tuple(blocks))
        plan.append(tuple(planb))
    return tuple(plan), tiles


def _prepare_in_maps(hidden_states, attention_mask, position_ids,
                     Wq, Wk, Wv, Wo):
    bf = ml_dtypes.bfloat16
    X = np.asarray(hidden_states, dtype=np.float32).reshape(T, H)
    XT = np.ascontiguousarray(X.T).astype(bf)  # [H, T]
    cosq, ssq, cosk, ssk, trig_len = _rope_trig(position_ids)
    plan, tiles = _classify_mask(attention_mask)
    n_uniq = len(tiles)
    masks = (np.stack(tiles) if n_uniq
             else np.zeros((1, 128, 512), bf)).astype(bf)
    trimask = np.where(np.arange(128)[None, :] >= np.arange(128)[:, None],
                       0.0, NEG).astype(bf)
    onescol = np.ones((128, 1), np.float32).astype(bf)
    onesrow = np.ones((2, 128), np.float32).astype(bf)
    ident = np.eye(128, dtype=np.float32).astype(bf)
    Wq = np.asarray(Wq, dtype=np.float32)
    Wk = np.asarray(Wk, dtype=np.float32)
    Wv = np.asarray(Wv, dtype=np.float32)
    Wo = np.asarray(Wo, dtype=np.float32)
    in_maps = []
    for c in range(NC):
        wkv = np.concatenate([Wk[:, c * D:(c + 1) * D],
                              Wv[:, c * D:(c + 1) * D]], axis=1)
        in_maps.append({
            "xt": XT,
            "wq": np.ascontiguousarray(
                Wq[:, c * QCOLS:(c + 1) * QCOLS]).astype(bf),
            "wkv": np.ascontiguousarray(wkv).astype(bf),
            "wo": np.ascontiguousarray(
                Wo[c * QCOLS:(c + 1) * QCOLS, :]).astype(bf),
            "cosq": cosq, "ssq": ssq, "cosk": cosk, "ssk": ssk,
            "masks": masks, "trimask": trimask,
            "onescol": onescol, "onesrow": onesrow, "ident": ident,
        })
    return in_maps, (plan, trig_len), n_uniq


# ---------------------------------------------------------------------------
# Execution (modeled on concourse.bass2jax.run_bass_via_pjrt)

_RUNNER_CACHE = {}


class SpmdRunner:
    def __init__(self, nc, n_cores=NC):
        import jax
        from jax.sharding import Mesh, PartitionSpec
        from jax.experimental.shard_map import shard_map
        from concourse.bass2jax import (_bass_exec_p, install_neuronx_cc_hook,
                                        partition_id_tensor)
        self.jax = jax
        install_neuronx_cc_hook()
        self.n_cores = n_cores
        partition_name = (nc.partition_id_tensor.name
                          if nc.partition_id_tensor else None)
        in_names, out_names, out_avals = [], [], []
        for alloc in nc.m.functions[0].allocations:
            if not isinstance(alloc, mybir.MemoryLocationSet):
                continue
            name = alloc.memorylocations[0].name
            if alloc.kind == "ExternalInput":
                in_names.append(name)
            elif alloc.kind == "ExternalOutput":
                out_names.append(name)
                out_avals.append(jax.core.ShapedArray(
                    tuple(alloc.tensor_shape), mybir.dt.np(alloc.dtype)))
        if partition_name is not None and partition_name in in_names:
            in_names.remove(partition_name)
        self.in_names, self.out_names, self.out_avals = \
            in_names, out_names, out_avals
        n_params, n_outs = len(in_names), len(out_avals)
        all_in_names = tuple(in_names + out_names)
        if partition_name is not None:
            all_in_names = all_in_names + (partition_name,)

        def _body(*args):
            operands = list(args)
            if partition_name is not None:
                operands.append(partition_id_tensor())
            outs = _bass_exec_p.bind(
                *operands, out_avals=tuple(out_avals), in_names=all_in_names,
                out_names=tuple(out_names), lowering_input_output_aliases=(),
                sim_require_finite=True, sim_require_nnan=True, nc=nc)
            return tuple(outs)

        devices = jax.devices()[:n_cores]
        mesh = Mesh(np.asarray(devices), ("core",))
        in_specs = (PartitionSpec("core"),) * (n_params + n_outs)
        out_specs = (PartitionSpec("core"),) * n_outs
        self.fn = jax.jit(
            shard_map(_body, mesh=mesh, in_specs=in_specs,
                      out_specs=out_specs, check_rep=False),
            keep_unused=True)

    def prepare(self, in_maps):
        concat_in = [
            np.concatenate([np.asarray(in_maps[c][name])
                            for c in range(self.n_cores)], axis=0)
            for name in self.in_names
        ]
        concat_zeros = [
            np.zeros((self.n_cores * a.shape[0], *a.shape[1:]), a.dtype)
            for a in self.out_avals
        ]
        return [self.jax.device_put(a) for a in concat_in + concat_zeros]

    def run(self, args):
        outs = self.fn(*args)
        self.jax.block_until_ready(outs)
        return outs

    def results(self, outs):
        return [
            {name: np.asarray(outs[i]).reshape(
                self.n_cores, *self.out_avals[i].shape)[c]
             for i, name in enumerate(self.out_names)}
            for c in range(self.n_cores)
        ]


def get_runner(plan_key, n_uniq, nreps=1):
    plan, trig_len = plan_key
    key = (plan, trig_len, n_uniq, nreps)
    r = _RUNNER_CACHE.get(key)
    if r is None:
        nc = _build_program(plan, n_uniq, nreps, trig_len=trig_len)
        r = SpmdRunner(nc)
        _RUNNER_CACHE[key] = r
    return r


def kernel(hidden_states, attention_mask, position_ids, Wq, Wk, Wv, Wo):
    in_maps, plan_key, n_uniq = _prepare_in_maps(
        hidden_states, attention_mask, position_ids, Wq, Wk, Wv, Wo)
    r = get_runner(plan_key, n_uniq, nreps=1)
    outs = r.run(r.prepare(in_maps))
    res = r.results(outs)
    full = res[0]["out"].astype(np.float32).copy()
    for c in range(1, NC):
        full += res[c]["out"]
    return full.reshape(B, S, H)
